# revision 18
# baseline (speedup 1.0000x reference)
"""GAT (graph attention) layer on 8 TRN2 NeuronCores — v3.

Algorithm (mathematically equal to the reference):
  proj = in_feat @ W_proj;  src_s = proj @ a_src;  tau = proj @ a_tgt
  per edge e=(s,t):  score_e = exp(leakyrelu(src_s[s] + tau[t]) - SHIFT)
  out[t] = (sum_e score_e * proj[s]) / (sum_e score_e) + bias
The reference's global-max shift is replaced by the constant SHIFT=16
(numerator/denominator scale identically).  exp(leakyrelu(x) - S) is
computed as max(exp(x-S), exp(0.2x-S)) — two ACT exps + one DVE max.

Sharding: edges sharded by TARGET node; each core owns a disjoint output
slice, no collectives.  Per core, targets are packed into 128-target
blocks; each block's segment sums (softmax denominator + weighted
feature sum) accumulate in PSUM via one-hot matmuls over edge tiles.

Pipeline structure (cost-model-driven):
 - An early cheap SCORE pass (x @ [W a_src | W a_tgt]) fills the score
   table first, unblocking every block's score/tau/S gathers and the
   exp-score chain while the expensive projection passes still run.
 - The projection runs as two passes (node tiles 0..195 -> pt_a,
   196..390 -> pt_b) with per-block edge segments split by source pass;
   a block's A-segment weighted features and one-hot matmuls proceed
   after pass A, overlapping pass B.
 - Per-block work is emitted with a LAG between early stages and
   table-dependent late stages so the in-order Pool queue never parks
   early gathers behind blocked ones.
 - All gathers/bulk loads declare int64 APs (gather cost is free-size
   elems x 0.833ns) and bulk loads are sequential-index gathers (far
   cheaper than dma_start on the shared DMA-engine resource).
 - One-hot S rows come from a constant identity table via gather.
"""
import sys
sys.path.insert(0, "/opt/trn_rl_repo")
import numpy as np

import concourse.bass as bass
import concourse.bacc as bacc
import concourse.mybir as mybir
import concourse.tile as tile
from concourse._compat import cdiv

P = 128
N_NODES = 50000
N_CORES = 8
D = 128
H = 4
NT = cdiv(N_NODES, P)               # 391 node tiles
NPAD = NT * P                       # 50048
NT_A = 196                          # pass-A node tiles (pt_a)
NT_B = NT - NT_A                    # 195
SHIFT = 16.0
EPS = 1e-16
PSPLIT = 64                         # partition split for score-table idx
ROWS_SLO = PSPLIT * NT              # score table halves
ROWS_SHI = (P - PSPLIT) * NT
XSLAB = 49                          # node tiles per x-slab input
NXS = cdiv(NT, XSLAB)               # 8
PK = 4                              # node tiles per psum group
WSLAB = 49                          # node tiles per table write
LAG = 4                             # blocks staged ahead of late stages

_cache = {}

CFG = {
    "acc_bufs": 4,
    "g_bufs": 2,
    "wk_bufs": 3,
    "sw_bufs": LAG + 1,
}


def _build(nb_lo, nb_hi, ks, with_bias):
    nc = bacc.Bacc("TRN2", target_bir_lowering=False, debug=False)
    f32, bf16 = mybir.dt.float32, mybir.dt.bfloat16
    i16, i64 = mybir.dt.int16, mybir.dt.int64

    NBLK = nb_lo + nb_hi
    k_alo, k_ahi, k_blo, k_bhi = ks
    k_a = k_alo + k_ahi                 # pass-A edge tiles per block
    k_b = k_blo + k_bhi
    T_B = k_a + k_b
    NIDX = T_B * P
    IW = T_B * 8
    IWPAD = cdiv(NBLK * IW * 2, 256) * 128

    xs_d = [nc.dram_tensor(f"xs{i}", [P, XSLAB * P], bf16, kind="ExternalInput")
            for i in range(NXS)]
    W_d = nc.dram_tensor("W", [P, D], bf16, kind="ExternalInput")
    WA_d = nc.dram_tensor("WA", [P, 8], bf16, kind="ExternalInput")
    ident_d = nc.dram_tensor("ident", [144, 32], i64, kind="ExternalInput")
    pidx_d = nc.dram_tensor("pidx", [P, IWPAD], i16, kind="ExternalInput")
    cidx_d = nc.dram_tensor("cidx", [P, IWPAD], i16, kind="ExternalInput")
    tidx_d = nc.dram_tensor("tidx", [P, IWPAD], i16, kind="ExternalInput")
    sidx_d = nc.dram_tensor("sidx", [P, IWPAD], i16, kind="ExternalInput")
    seq_d = nc.dram_tensor("seq", [P, 16], i16, kind="ExternalInput")
    if with_bias:
        bias_d = nc.dram_tensor("bias", [1, D], f32, kind="ExternalInput")
    out_d = nc.dram_tensor("out", [NBLK * P, D], f32, kind="ExternalOutput")

    # tables: pt_a/pt_b proj rows (256B) at row p*NT_A+nt / p*NT_B+(nt-NT_A);
    # st score rows (256B stride, [src_s 4f32 | tau 4f32] written), p-split.
    pt_a = nc.dram_tensor("pt_a", [P * NT_A, 32], i64)
    pt_b = nc.dram_tensor("pt_b", [P * NT_B, 32], i64)
    st_lo = nc.dram_tensor("st_lo", [ROWS_SLO, 32], i64)
    st_hi = nc.dram_tensor("st_hi", [ROWS_SHI, 32], i64)

    with tile.TileContext(nc) as tc:
        with (
            tc.tile_pool(name="const", bufs=1) as cp,
            tc.tile_pool(name="p1x", bufs=2) as p1x,
            tc.tile_pool(name="p1w", bufs=2) as p1w,
            tc.tile_pool(name="p1ps", bufs=2, space="PSUM") as p1ps,
            tc.tile_pool(name="g", bufs=CFG["g_bufs"]) as g,
            tc.tile_pool(name="sw", bufs=CFG["sw_bufs"]) as sw,
            tc.tile_pool(name="wk", bufs=CFG["wk_bufs"]) as wk,
            tc.tile_pool(name="acc", bufs=CFG["acc_bufs"], space="PSUM") as accp,
            tc.tile_pool(name="ep", bufs=2) as ep,
        ):
            from concourse.library_config import mlp
            nc.gpsimd.load_library(mlp)

            seq = cp.tile([P, 16], i16)
            nc.sync.dma_start(seq[:], seq_d[:])
            W_sb = cp.tile([P, D], bf16)
            nc.sync.dma_start(W_sb[:], W_d[:])
            WA_sb = cp.tile([P, 8], bf16)
            nc.sync.dma_start(WA_sb[:], WA_d[:])
            nshift = cp.tile([P, 1], f32)
            nc.gpsimd.memset(nshift[:], -SHIFT)
            sc02 = cp.tile([P, 1], f32)
            nc.gpsimd.memset(sc02[:], 0.2)

            def bulk_load(dst_ap, src_t, n_i64):
                nc.gpsimd.dma_gather(
                    dst_ap.rearrange("p (k c) -> p k c", k=1),
                    src_t, seq[:, :8], P, P, n_i64, single_packet=False)

            pidx = cp.tile([P, IWPAD], i16)
            cidx = cp.tile([P, IWPAD], i16)
            tidx = cp.tile([P, IWPAD], i16)
            sidx = cp.tile([P, IWPAD], i16)
            bulk_load(pidx[:].bitcast(i64), pidx_d[:].bitcast(i64), IWPAD // 4)
            bulk_load(cidx[:].bitcast(i64), cidx_d[:].bitcast(i64), IWPAD // 4)
            bulk_load(tidx[:].bitcast(i64), tidx_d[:].bitcast(i64), IWPAD // 4)
            bulk_load(sidx[:].bitcast(i64), sidx_d[:].bitcast(i64), IWPAD // 4)
            if with_bias:
                ones_row = cp.tile([1, P], f32)
                nc.gpsimd.memset(ones_row[:], 1.0)
                bias_row = cp.tile([1, D], f32)
                nc.sync.dma_start(bias_row[:], bias_d[:])
                bias_ps = accp.tile([P, D], f32, tag="init")
                nc.tensor.matmul(out=bias_ps[:], lhsT=ones_row[:], rhs=bias_row[:],
                                 start=True, stop=True)
                bias_mat = cp.tile([P, D], f32)
                nc.vector.tensor_copy(out=bias_mat[:], in_=bias_ps[:])

            def load_slab(i):
                xt = p1x.tile([P, XSLAB * P], bf16, tag="xs")
                bulk_load(xt[:].bitcast(i64), xs_d[i][:].bitcast(i64),
                          XSLAB * P // 4)
                return xt

            # ---------- score pass: scores -> st tables ----------
            xs = [load_slab(i) for i in range(NXS)]
            for ws in range(NXS):      # one wslab == one x slab (49 tiles)
                base = ws * WSLAB
                w = min(WSLAB, NT - base)
                srow = p1w.tile([P, WSLAB * 8], f32, tag="srow")
                srow_r = srow[:].rearrange("p (j c) -> p j c", j=WSLAB)
                for g0 in range(0, w, PK):
                    k = min(PK, w - g0)
                    ps = p1ps.tile([P, PK * 256], f32, tag="ps")
                    for j in range(k):
                        nt = base + g0 + j
                        o = (nt % XSLAB) * P
                        nc.tensor.matmul(out=ps[:, j * 256:j * 256 + 8],
                                         lhsT=xs[ws][:, o:o + P],
                                         rhs=WA_sb[:], start=True, stop=True)
                    ps_r = ps[:].rearrange("p (j c) -> p j c", j=PK)[:, :k, :]
                    nc.vector.tensor_copy(
                        out=srow_r[:, g0:g0 + k, :], in_=ps_r[:, :, 0:8])
                sr = srow_r[:, :w, :]
                nc.sync.dma_start(
                    st_lo[:].bitcast(f32).rearrange(
                        "(p nt) c -> p nt c", p=PSPLIT)[:, base:base + w, 0:8],
                    sr[0:PSPLIT])
                nc.sync.dma_start(
                    st_hi[:].bitcast(f32).rearrange(
                        "(p nt) c -> p nt c", p=P - PSPLIT)[:, base:base + w, 0:8],
                    sr[PSPLIT:P])

            # ---------- block stages ----------
            def early(b):
                """score/tau/S gathers + exp-score chain -> wide, S staged."""
                st_t = st_lo if b < nb_lo else st_hi
                gi0 = b * IW
                srcs = g.tile([P, T_B * 32], i64, tag="gsrc")
                taut = g.tile([P, T_B * 32], i64, tag="gtau")
                Sg = sw.tile([P, T_B * 32], i64, tag="gS")
                wide = sw.tile([P, T_B * (D + H)], bf16, tag="wide")
                segs = ((0, k_alo, st_lo), (k_alo, k_ahi, st_hi),
                        (k_a, k_blo, st_lo), (k_a + k_blo, k_bhi, st_hi))
                for (o, kk, st_s) in segs:
                    if kk == 0:
                        continue
                    nc.gpsimd.dma_gather(
                        srcs[:, o * 32:(o + kk) * 32]
                            .rearrange("p (k c) -> p k c", k=kk),
                        st_s[:], cidx[:, gi0 + o * 8:gi0 + (o + kk) * 8],
                        kk * P, kk * P, 32, single_packet=False)
                nc.gpsimd.dma_gather(
                    taut[:].rearrange("p (k c) -> p k c", k=T_B),
                    st_t[:], tidx[:, gi0:gi0 + IW],
                    NIDX, NIDX, 32, single_packet=False)
                nc.gpsimd.dma_gather(
                    Sg[:].rearrange("p (k c) -> p k c", k=T_B),
                    ident_d[:], sidx[:, gi0:gi0 + IW],
                    NIDX, NIDX, 32, single_packet=False)

                srcs_f = srcs[:].bitcast(f32).rearrange("p (j c) -> p j c", j=T_B)
                taut_f = taut[:].bitcast(f32).rearrange("p (j c) -> p j c", j=T_B)
                xb = wk.tile([P, T_B * H], f32, tag="xb")
                xb_r = xb[:].rearrange("p (j h) -> p j h", j=T_B)
                nc.vector.tensor_tensor(
                    out=xb_r, in0=srcs_f[:, :, 0:H], in1=taut_f[:, :, H:2 * H],
                    op=mybir.AluOpType.add)
                wide_r = wide[:].rearrange("p (j c) -> p j c", j=T_B)
                e2 = wk.tile([P, T_B * H], bf16, tag="e2")
                e2_r = e2[:].rearrange("p (j h) -> p j h", j=T_B)
                nc.scalar.activation(
                    out=wide_r[:, :, D:], in_=xb_r,
                    func=mybir.ActivationFunctionType.Exp, bias=nshift[:])
                nc.scalar.activation(
                    out=e2_r, in_=xb_r,
                    func=mybir.ActivationFunctionType.Exp, bias=nshift[:],
                    scale=sc02[:])
                nc.vector.tensor_tensor(
                    out=wide_r[:, :, D:], in0=wide_r[:, :, D:], in1=e2_r,
                    op=mybir.AluOpType.max)
                return Sg, wide

            def late_half(b, Sg, wide, acc, which):
                """rows gather + weighted mult + one-hot matmuls, one pass."""
                gi0 = b * IW
                if which == 0:
                    pt, o, kk = pt_a, 0, k_a
                else:
                    pt, o, kk = pt_b, k_a, k_b
                if kk == 0:
                    return
                rows = g.tile([P, max(k_a, k_b) * 32], i64, tag=f"grow{which}")
                nc.gpsimd.dma_gather(
                    rows[:, :kk * 32].rearrange("p (k c) -> p k c", k=kk),
                    pt[:], pidx[:, gi0 + o * 8:gi0 + (o + kk) * 8],
                    kk * P, kk * P, 32, single_packet=False)
                wide_r = wide[:].rearrange("p (j c) -> p j c", j=T_B)
                nc.vector.tensor_tensor(
                    out=wide_r[:, o:o + kk, :D].rearrange(
                        "p j (r h) -> p j r h", h=H),
                    in0=rows[:, :kk * 32].bitcast(bf16)
                        .rearrange("p (j c) -> p j c", j=kk)
                        .rearrange("p j (r h) -> p j r h", h=H),
                    in1=wide_r[:, o:o + kk, D:].unsqueeze(2)
                        .to_broadcast([P, kk, 32, H]),
                    op=mybir.AluOpType.mult)
                Sg_b = Sg[:].bitcast(bf16).rearrange("p (j c) -> p j c", j=T_B)
                for j in range(o, o + kk):
                    nc.tensor.matmul(
                        out=acc[:], lhsT=Sg_b[:, j, :],
                        rhs=wide[:, j * (D + H):(j + 1) * (D + H)],
                        start=(j == 0), stop=(j == T_B - 1))

            def epilogue(b, acc):
                den = ep.tile([P, H], f32, tag="den")
                nc.scalar.activation(out=den[:], in_=acc[:, D:],
                                     func=mybir.ActivationFunctionType.Copy,
                                     bias=float(EPS))
                recip = ep.tile([P, H], f32, tag="recip")
                nc.vector.reciprocal(recip[:], den[:])
                out_sb = ep.tile([P, D], f32, tag="outsb")
                nc.vector.tensor_tensor(
                    out=out_sb[:].rearrange("p (h r) -> p r h", h=H),
                    in0=acc[:, :D].rearrange("p (r h) -> p r h", h=H),
                    in1=recip[:].unsqueeze(1).to_broadcast([P, 32, H]),
                    op=mybir.AluOpType.mult)
                if with_bias:
                    nc.vector.tensor_tensor(
                        out=out_sb[:], in0=out_sb[:], in1=bias_mat[:],
                        op=mybir.AluOpType.add)
                nc.sync.dma_start(out_d[b * P:(b + 1) * P, :], out_sb[:])

            def proj_pass(which, xslabs):
                nt0, ntn, pt = ((0, NT_A, pt_a) if which == 0
                                else (NT_A, NT, pt_b))
                for ws in range(nt0 // WSLAB, cdiv(ntn, WSLAB)):
                    base = ws * WSLAB
                    w = min(WSLAB, ntn - base)
                    xt = xslabs[ws]
                    prow = p1w.tile([P, WSLAB * D], bf16, tag="prow")
                    prow_r = prow[:].rearrange("p (j c) -> p j c", j=WSLAB)
                    for g0 in range(0, w, PK):
                        k = min(PK, w - g0)
                        ps = p1ps.tile([P, PK * 256], f32, tag="ps")
                        for j in range(k):
                            nt = base + g0 + j
                            o = (nt % XSLAB) * P
                            nc.tensor.matmul(out=ps[:, j * 256:j * 256 + D],
                                             lhsT=xt[:, o:o + P],
                                             rhs=W_sb[:], start=True, stop=True)
                        ps_r = ps[:].rearrange("p (j c) -> p j c", j=PK)[:, :k, :]
                        nc.scalar.activation(
                            out=prow_r[:, g0:g0 + k, :], in_=ps_r[:, :, 0:D],
                            func=mybir.ActivationFunctionType.Copy)
                    nc.gpsimd.dma_start(
                        pt[:].bitcast(bf16).rearrange(
                            "(p nt) c -> p nt c",
                            p=P)[:, base - nt0:base - nt0 + w, :],
                        prow_r[:, :w, :])

            # ---------- emission: earlies + pass A + lates + pass B ----------
            staged = {}
            for b in range(min(LAG, NBLK)):
                staged[b] = early(b)

            # x slabs were consumed by the score pass; reload for proj passes.
            xs2 = [load_slab(i) for i in range(NXS)]
            proj_pass(0, xs2)

            nxt = LAG
            for b in range(NBLK):
                Sg, wide = staged.pop(b)
                acc = accp.tile([P, D + H], f32, tag="acc")
                late_half(b, Sg, wide, acc, 0)
                if nxt < NBLK:
                    staged[nxt] = early(nxt)
                    nxt += 1
                staged[b] = (Sg, wide, acc)
                if b == LAG - 1:
                    proj_pass(1, xs2)
            for b in range(NBLK):
                Sg, wide, acc = staged.pop(b)
                late_half(b, Sg, wide, acc, 1)
                epilogue(b, acc)

    nc.compile()
    return nc


def _wrap16(seg):
    """dma_gather idx layout: entry i at [i%16, i//16], replicated x8."""
    n = len(seg)
    w = seg.reshape(n // 16, 16).T
    return np.tile(w, (8, 1))


def _prep_host(in_feat, edge_ind, W_proj, a_src, a_tgt, bias):
    import ml_dtypes
    bfd = ml_dtypes.bfloat16
    src = np.asarray(edge_ind[0]).astype(np.int64)
    tgt = np.asarray(edge_ind[1]).astype(np.int64)
    x = np.asarray(in_feat, np.float32)
    W = np.asarray(W_proj, np.float32)
    a_src = np.asarray(a_src, np.float32).reshape(H, 32)
    a_tgt = np.asarray(a_tgt, np.float32).reshape(H, 32)
    bias = np.asarray(bias, np.float32).reshape(-1)

    # W head-interleaved (col r*4+h); WA = [W@a_src_h | W@a_tgt_h]
    Wb = W.astype(bfd).astype(np.float32)
    perm = np.arange(D).reshape(H, 32).T.reshape(-1)
    W_in = Wb[:, perm]
    WA = np.zeros((P, 8), np.float32)
    for h in range(H):
        sel = np.zeros((D,), np.float32)
        sel[h * 32:(h + 1) * 32] = a_src[h]
        WA[:, h] = Wb @ sel
        sel = np.zeros((D,), np.float32)
        sel[h * 32:(h + 1) * 32] = a_tgt[h]
        WA[:, H + h] = Wb @ sel

    xT = np.zeros((P, NPAD), np.float32)
    xT[:, :N_NODES] = x.T
    xs_in = {}
    for i in range(NXS):
        sl = np.zeros((P, XSLAB * P), bfd)
        w = min(XSLAB * P, NPAD - i * XSLAB * P)
        sl[:, :w] = xT[:, i * XSLAB * P:i * XSLAB * P + w].astype(bfd)
        xs_in[f"xs{i}"] = sl

    ident = np.zeros((144, P), bfd)
    for q in range(P):
        ident[q, q] = 1.0

    # ---- edge partitioning ----
    # target shard: tile-aligned per-core ranges
    tile_core = np.minimum((np.arange(NT) * N_CORES) // NT, N_CORES - 1)
    core_of_node = tile_core[np.arange(N_NODES) // P]
    core = core_of_node[tgt]
    p_of_t = tgt % P
    t_is_lo = p_of_t < PSPLIT
    src_is_lo = (src % P) < PSPLIT
    src_is_a = (src // P) < NT_A

    deg_a = np.bincount(tgt[src_is_a], minlength=N_NODES)
    deg_b = np.bincount(tgt[~src_is_a], minlength=N_NODES)
    blk_of = np.full(N_NODES, -1, np.int32)
    tin_of = np.zeros(N_NODES, np.int32)
    nb_lo = nb_hi = 0
    for c in range(N_CORES):
        ids_all = np.nonzero(core_of_node == c)[0]
        nb_lo = max(nb_lo, cdiv(int(((ids_all % P) < PSPLIT).sum()), P))
        nb_hi = max(nb_hi, cdiv(int(((ids_all % P) >= PSPLIT).sum()), P))
    for c in range(N_CORES):
        ids_all = np.nonzero(core_of_node == c)[0]
        for half, nb, b0 in ((0, nb_lo, 0), (1, nb_hi, nb_lo)):
            sel = (ids_all % P) < PSPLIT if half == 0 else (ids_all % P) >= PSPLIT
            ids = ids_all[sel]
            order = np.argsort(-(deg_a[ids] + deg_b[ids]), kind="stable")
            loads_a = np.zeros(nb, np.int64)
            loads_b = np.zeros(nb, np.int64)
            fill = np.zeros(nb, np.int32)
            for t in ids[order]:
                cand = np.nonzero(fill < P)[0]
                j = cand[np.argmin(np.maximum(loads_a[cand] + deg_a[t],
                                              loads_b[cand] + deg_b[t])
                                   + 0.001 * fill[cand])]
                blk_of[t] = b0 + j
                tin_of[t] = fill[j]
                fill[j] += 1
                loads_a[j] += deg_a[t]
                loads_b[j] += deg_b[t]
    NBLK = nb_lo + nb_hi
    blk = blk_of[tgt]
    tin = tin_of[tgt]

    # per (core, block) per segment (A-lo, A-hi, B-lo, B-hi) tile counts
    seg_of = (~src_is_a).astype(np.int64) * 2 + (~src_is_lo).astype(np.int64)
    key = (core * NBLK + blk) * 4 + seg_of
    seg_n = np.bincount(key, minlength=N_CORES * NBLK * 4).reshape(-1, 4)
    ks = tuple(max(1, cdiv(int(seg_n[:, s].max()), P)) for s in range(4))
    k_alo, k_ahi, k_blo, k_bhi = ks
    T_B = sum(ks)
    IW = T_B * 8
    IWPAD = cdiv(NBLK * IW * 2, 256) * 128

    # row ids: proj tables by pass; score tables by p-half.
    prow_id = np.where(src_is_a, (src % P) * NT_A + src // P,
                       (src % P) * NT_B + (src // P - NT_A))
    crow_id = (src % P - np.where(src_is_lo, 0, PSPLIT)) * NT + src // P
    trow_id = (tgt % P - np.where(t_is_lo, 0, PSPLIT)) * NT + tgt // P

    seq = _wrap16(np.concatenate([np.arange(P, dtype=np.int16),
                                  np.zeros(P, np.int16)]))[:, :16]
    with_bias = bool(np.any(bias != 0.0))
    shared = {**xs_in, "W": W_in.astype(bfd), "WA": WA.astype(bfd),
              "ident": ident.view(np.int64), "seq": seq}
    if with_bias:
        shared["bias"] = bias.reshape(1, D)

    bounds = np.cumsum([0, k_alo, k_ahi, k_blo, k_bhi]) * P
    core_inputs = []
    out_perm = np.full((N_CORES, NBLK * P), -1, np.int64)
    for c in range(N_CORES):
        ids_all = np.nonzero(core_of_node == c)[0]
        for t in ids_all:
            out_perm[c, blk_of[t] * P + tin_of[t]] = t
        m = core == c
        cb = blk[m]
        cseg = seg_of[m]
        cp_, cc, ct_, cti = prow_id[m], crow_id[m], trow_id[m], tin[m]
        pidx = np.zeros((NBLK, T_B * P), np.int16)
        c16 = np.zeros((NBLK, T_B * P), np.int16)
        t16 = np.zeros((NBLK, T_B * P), np.int16)
        s16 = np.full((NBLK, T_B * P), 128, np.int16)
        for b in range(NBLK):
            mb = cb == b
            for s0 in range(4):
                ms = mb & (cseg == s0)
                n = int(ms.sum())
                o = int(bounds[s0])
                pidx[b, o:o + n] = cp_[ms].astype(np.int16)
                c16[b, o:o + n] = cc[ms].astype(np.int16)
                t16[b, o:o + n] = ct_[ms].astype(np.int16)
                s16[b, o:o + n] = cti[ms].astype(np.int16)
        pw = np.zeros((P, IWPAD), np.int16)
        cw = np.zeros((P, IWPAD), np.int16)
        tw = np.zeros((P, IWPAD), np.int16)
        sw_ = np.zeros((P, IWPAD), np.int16)
        for b in range(NBLK):
            for s0 in range(4):
                lo, hi = bounds[s0], bounds[s0 + 1]
                pw[:, b * IW + lo // 16:b * IW + hi // 16] = \
                    _wrap16(pidx[b, lo:hi])
                cw[:, b * IW + lo // 16:b * IW + hi // 16] = \
                    _wrap16(c16[b, lo:hi])
            tw[:, b * IW:(b + 1) * IW] = _wrap16(t16[b])
            sw_[:, b * IW:(b + 1) * IW] = _wrap16(s16[b])
        core_inputs.append({**shared, "pidx": pw, "cidx": cw,
                            "tidx": tw, "sidx": sw_})
    return (nb_lo, nb_hi, ks, with_bias), core_inputs, out_perm


def kernel(in_feat, edge_ind, edge_len, W_proj, a_src, a_tgt, bias):
    kkey, core_inputs, out_perm = _prep_host(in_feat, edge_ind, W_proj,
                                             a_src, a_tgt, bias)
    if kkey not in _cache:
        _cache[kkey] = _build(*kkey)
    nc = _cache[kkey]

    from concourse.bass_utils import run_bass_kernel_spmd
    res = run_bass_kernel_spmd(nc, core_inputs, list(range(N_CORES)))

    out = np.zeros((N_NODES, D), np.float32)
    for c in range(N_CORES):
        o = res.results[c]["out"]
        valid = out_perm[c] >= 0
        out[out_perm[c][valid]] = o[valid]
    return out


# revision 19
# speedup vs baseline: 1.0281x; 1.0281x over previous
"""GAT (graph attention) layer on 8 TRN2 NeuronCores — v3.

Algorithm (mathematically equal to the reference):
  proj = in_feat @ W_proj;  src_s = proj @ a_src;  tau = proj @ a_tgt
  per edge e=(s,t):  score_e = exp(leakyrelu(src_s[s] + tau[t]) - SHIFT)
  out[t] = (sum_e score_e * proj[s]) / (sum_e score_e) + bias
The reference's global-max shift is replaced by the constant SHIFT=16
(numerator/denominator scale identically).  exp(leakyrelu(x) - S) is
computed as max(exp(x-S), exp(0.2x-S)) — two ACT exps + one DVE max.

Sharding: edges sharded by TARGET node; each core owns a disjoint output
slice, no collectives.  Per core, targets are packed into 128-target
blocks; each block's segment sums (softmax denominator + weighted
feature sum) accumulate in PSUM via one-hot matmuls over edge tiles.

Pipeline structure (cost-model-driven):
 - An early cheap SCORE pass (x @ [W a_src | W a_tgt]) fills the score
   table first, unblocking every block's score/tau/S gathers and the
   exp-score chain while the expensive projection passes still run.
 - The projection runs as two passes (node tiles 0..195 -> pt_a,
   196..390 -> pt_b) with per-block edge segments split by source pass;
   a block's A-segment weighted features and one-hot matmuls proceed
   after pass A, overlapping pass B.
 - Per-block work is emitted with a LAG between early stages and
   table-dependent late stages so the in-order Pool queue never parks
   early gathers behind blocked ones.
 - All gathers/bulk loads declare int64 APs (gather cost is free-size
   elems x 0.833ns) and bulk loads are sequential-index gathers (far
   cheaper than dma_start on the shared DMA-engine resource).
 - One-hot S rows come from a constant identity table via gather.
"""
import sys
sys.path.insert(0, "/opt/trn_rl_repo")
import numpy as np

import concourse.bass as bass
import concourse.bacc as bacc
import concourse.mybir as mybir
import concourse.tile as tile
from concourse._compat import cdiv

P = 128
N_NODES = 50000
N_CORES = 8
D = 128
H = 4
NT = cdiv(N_NODES, P)               # 391 node tiles
NPAD = NT * P                       # 50048
NT_A = 196                          # pass-A node tiles (pt_a)
NT_B = NT - NT_A                    # 195
SHIFT = 16.0
EPS = 1e-16
PSPLIT = 64                         # partition split for score-table idx
ROWS_SLO = PSPLIT * NT              # score table halves
ROWS_SHI = (P - PSPLIT) * NT
XSLAB = 49                          # node tiles per x-slab input
NXS = cdiv(NT, XSLAB)               # 8
PK = 4                              # node tiles per psum group
WSLAB = 49                          # node tiles per table write
LAG = 4                             # blocks staged ahead of late stages

_cache = {}

CFG = {
    "acc_bufs": 4,
    "g_bufs": 2,
    "wk_bufs": 3,
    "sw_bufs": LAG + 1,
}


def _build(nb_lo, nb_hi, ks, with_bias):
    nc = bacc.Bacc("TRN2", target_bir_lowering=False, debug=False)
    f32, bf16 = mybir.dt.float32, mybir.dt.bfloat16
    i16, i64 = mybir.dt.int16, mybir.dt.int64

    NBLK = nb_lo + nb_hi
    k_alo, k_ahi, k_blo, k_bhi = ks
    k_a = k_alo + k_ahi                 # pass-A edge tiles per block
    k_b = k_blo + k_bhi
    T_B = k_a + k_b
    NIDX = T_B * P
    IW = T_B * 8
    IWPAD = cdiv(NBLK * IW * 2, 256) * 128

    xs_d = [nc.dram_tensor(f"xs{i}", [P, XSLAB * P], bf16, kind="ExternalInput")
            for i in range(NXS)]
    W_d = nc.dram_tensor("W", [P, D], bf16, kind="ExternalInput")
    WA_d = nc.dram_tensor("WA", [P, 8], bf16, kind="ExternalInput")
    ident_d = nc.dram_tensor("ident", [144, 32], i64, kind="ExternalInput")
    pidx_d = nc.dram_tensor("pidx", [P, IWPAD], i16, kind="ExternalInput")
    cidx_d = nc.dram_tensor("cidx", [P, IWPAD], i16, kind="ExternalInput")
    tidx_d = nc.dram_tensor("tidx", [P, IWPAD], i16, kind="ExternalInput")
    sidx_d = nc.dram_tensor("sidx", [P, IWPAD], i16, kind="ExternalInput")
    seq_d = nc.dram_tensor("seq", [P, 16], i16, kind="ExternalInput")
    if with_bias:
        bias_d = nc.dram_tensor("bias", [1, D], f32, kind="ExternalInput")
    out_d = nc.dram_tensor("out", [NBLK * P, D], f32, kind="ExternalOutput")

    # tables: pt_a/pt_b proj rows (256B) at row p*NT_A+nt / p*NT_B+(nt-NT_A);
    # st score rows (256B stride, [src_s 4f32 | tau 4f32] written), p-split.
    pt_a = nc.dram_tensor("pt_a", [P * NT_A, 32], i64)
    pt_b = nc.dram_tensor("pt_b", [P * NT_B, 32], i64)
    st_lo = nc.dram_tensor("st_lo", [ROWS_SLO, 32], i64)
    st_hi = nc.dram_tensor("st_hi", [ROWS_SHI, 32], i64)

    with tile.TileContext(nc) as tc:
        with (
            tc.tile_pool(name="const", bufs=1) as cp,
            tc.tile_pool(name="p1x", bufs=2) as p1x,
            tc.tile_pool(name="p1w", bufs=2) as p1w,
            tc.tile_pool(name="p1ps", bufs=2, space="PSUM") as p1ps,
            tc.tile_pool(name="g", bufs=CFG["g_bufs"]) as g,
            tc.tile_pool(name="sw", bufs=CFG["sw_bufs"]) as sw,
            tc.tile_pool(name="wk", bufs=CFG["wk_bufs"]) as wk,
            tc.tile_pool(name="acc", bufs=CFG["acc_bufs"], space="PSUM") as accp,
            tc.tile_pool(name="ep", bufs=2) as ep,
        ):
            from concourse.library_config import mlp
            nc.gpsimd.load_library(mlp)

            seq = cp.tile([P, 16], i16)
            nc.sync.dma_start(seq[:], seq_d[:])
            W_sb = cp.tile([P, D], bf16)
            nc.sync.dma_start(W_sb[:], W_d[:])
            WA_sb = cp.tile([P, 8], bf16)
            nc.sync.dma_start(WA_sb[:], WA_d[:])
            nshift = cp.tile([P, 1], f32)
            nc.gpsimd.memset(nshift[:], -SHIFT)
            sc02 = cp.tile([P, 1], f32)
            nc.gpsimd.memset(sc02[:], 0.2)

            def bulk_load(dst_ap, src_t, n_i64):
                nc.gpsimd.dma_gather(
                    dst_ap.rearrange("p (k c) -> p k c", k=1),
                    src_t, seq[:, :8], P, P, n_i64, single_packet=False)

            pidx = cp.tile([P, IWPAD], i16)
            cidx = cp.tile([P, IWPAD], i16)
            tidx = cp.tile([P, IWPAD], i16)
            sidx = cp.tile([P, IWPAD], i16)
            bulk_load(pidx[:].bitcast(i64), pidx_d[:].bitcast(i64), IWPAD // 4)
            bulk_load(cidx[:].bitcast(i64), cidx_d[:].bitcast(i64), IWPAD // 4)
            bulk_load(tidx[:].bitcast(i64), tidx_d[:].bitcast(i64), IWPAD // 4)
            bulk_load(sidx[:].bitcast(i64), sidx_d[:].bitcast(i64), IWPAD // 4)
            if with_bias:
                ones_row = cp.tile([1, P], f32)
                nc.gpsimd.memset(ones_row[:], 1.0)
                bias_row = cp.tile([1, D], f32)
                nc.sync.dma_start(bias_row[:], bias_d[:])
                bias_ps = accp.tile([P, D], f32, tag="init")
                nc.tensor.matmul(out=bias_ps[:], lhsT=ones_row[:], rhs=bias_row[:],
                                 start=True, stop=True)
                bias_mat = cp.tile([P, D], f32)
                nc.vector.tensor_copy(out=bias_mat[:], in_=bias_ps[:])

            def load_slab(i):
                xt = p1x.tile([P, XSLAB * P], bf16, tag="xs")
                bulk_load(xt[:].bitcast(i64), xs_d[i][:].bitcast(i64),
                          XSLAB * P // 4)
                return xt

            # ---------- score pass: scores -> st tables ----------
            xs = [load_slab(i) for i in range(NXS)]
            for ws in range(NXS):      # one wslab == one x slab (49 tiles)
                base = ws * WSLAB
                w = min(WSLAB, NT - base)
                srow = p1w.tile([P, WSLAB * 8], f32, tag="srow")
                srow_r = srow[:].rearrange("p (j c) -> p j c", j=WSLAB)
                for g0 in range(0, w, PK):
                    k = min(PK, w - g0)
                    ps = p1ps.tile([P, PK * 256], f32, tag="ps")
                    for j in range(k):
                        nt = base + g0 + j
                        o = (nt % XSLAB) * P
                        nc.tensor.matmul(out=ps[:, j * 256:j * 256 + 8],
                                         lhsT=xs[ws][:, o:o + P],
                                         rhs=WA_sb[:], start=True, stop=True)
                    ps_r = ps[:].rearrange("p (j c) -> p j c", j=PK)[:, :k, :]
                    nc.vector.tensor_copy(
                        out=srow_r[:, g0:g0 + k, :], in_=ps_r[:, :, 0:8])
                sr = srow_r[:, :w, :]
                nc.sync.dma_start(
                    st_lo[:].bitcast(f32).rearrange(
                        "(p nt) c -> p nt c", p=PSPLIT)[:, base:base + w, 0:8],
                    sr[0:PSPLIT])
                nc.sync.dma_start(
                    st_hi[:].bitcast(f32).rearrange(
                        "(p nt) c -> p nt c", p=P - PSPLIT)[:, base:base + w, 0:8],
                    sr[PSPLIT:P])

            # ---------- block stages ----------
            def early(b):
                """score/tau/S gathers + exp-score chain -> wide, S staged."""
                st_t = st_lo if b < nb_lo else st_hi
                gi0 = b * IW
                srcs = g.tile([P, T_B * 32], i64, tag="gsrc")
                taut = g.tile([P, T_B * 32], i64, tag="gtau")
                Sg = sw.tile([P, T_B * 32], i64, tag="gS")
                wide = sw.tile([P, T_B * (D + H)], bf16, tag="wide")
                segs = ((0, k_alo, st_lo), (k_alo, k_ahi, st_hi),
                        (k_a, k_blo, st_lo), (k_a + k_blo, k_bhi, st_hi))
                for (o, kk, st_s) in segs:
                    if kk == 0:
                        continue
                    nc.gpsimd.dma_gather(
                        srcs[:, o * 32:(o + kk) * 32]
                            .rearrange("p (k c) -> p k c", k=kk),
                        st_s[:], cidx[:, gi0 + o * 8:gi0 + (o + kk) * 8],
                        kk * P, kk * P, 32, single_packet=False)
                nc.gpsimd.dma_gather(
                    taut[:].rearrange("p (k c) -> p k c", k=T_B),
                    st_t[:], tidx[:, gi0:gi0 + IW],
                    NIDX, NIDX, 32, single_packet=False)
                nc.gpsimd.dma_gather(
                    Sg[:].rearrange("p (k c) -> p k c", k=T_B),
                    ident_d[:], sidx[:, gi0:gi0 + IW],
                    NIDX, NIDX, 32, single_packet=False)

                srcs_f = srcs[:].bitcast(f32).rearrange("p (j c) -> p j c", j=T_B)
                taut_f = taut[:].bitcast(f32).rearrange("p (j c) -> p j c", j=T_B)
                xb = wk.tile([P, T_B * H], f32, tag="xb")
                xb_r = xb[:].rearrange("p (j h) -> p j h", j=T_B)
                nc.vector.tensor_tensor(
                    out=xb_r, in0=srcs_f[:, :, 0:H], in1=taut_f[:, :, H:2 * H],
                    op=mybir.AluOpType.add)
                wide_r = wide[:].rearrange("p (j c) -> p j c", j=T_B)
                e2 = wk.tile([P, T_B * H], bf16, tag="e2")
                e2_r = e2[:].rearrange("p (j h) -> p j h", j=T_B)
                nc.scalar.activation(
                    out=wide_r[:, :, D:], in_=xb_r,
                    func=mybir.ActivationFunctionType.Exp, bias=nshift[:])
                nc.scalar.activation(
                    out=e2_r, in_=xb_r,
                    func=mybir.ActivationFunctionType.Exp, bias=nshift[:],
                    scale=sc02[:])
                nc.vector.tensor_tensor(
                    out=wide_r[:, :, D:], in0=wide_r[:, :, D:], in1=e2_r,
                    op=mybir.AluOpType.max)
                return Sg, wide

            def late_half(b, Sg, wide, acc, which):
                """rows gather + weighted mult + one-hot matmuls, one pass."""
                gi0 = b * IW
                if which == 0:
                    pt, o, kk = pt_a, 0, k_a
                else:
                    pt, o, kk = pt_b, k_a, k_b
                if kk == 0:
                    return
                rows = g.tile([P, max(k_a, k_b) * 32], i64, tag=f"grow{which}")
                nc.gpsimd.dma_gather(
                    rows[:, :kk * 32].rearrange("p (k c) -> p k c", k=kk),
                    pt[:], pidx[:, gi0 + o * 8:gi0 + (o + kk) * 8],
                    kk * P, kk * P, 32, single_packet=False)
                wide_r = wide[:].rearrange("p (j c) -> p j c", j=T_B)
                nc.vector.tensor_tensor(
                    out=wide_r[:, o:o + kk, :D].rearrange(
                        "p j (r h) -> p j r h", h=H),
                    in0=rows[:, :kk * 32].bitcast(bf16)
                        .rearrange("p (j c) -> p j c", j=kk)
                        .rearrange("p j (r h) -> p j r h", h=H),
                    in1=wide_r[:, o:o + kk, D:].unsqueeze(2)
                        .to_broadcast([P, kk, 32, H]),
                    op=mybir.AluOpType.mult)
                Sg_b = Sg[:].bitcast(bf16).rearrange("p (j c) -> p j c", j=T_B)
                for j in range(o, o + kk):
                    nc.tensor.matmul(
                        out=acc[:], lhsT=Sg_b[:, j, :],
                        rhs=wide[:, j * (D + H):(j + 1) * (D + H)],
                        start=(j == 0), stop=(j == T_B - 1))

            def epilogue(b, acc):
                den = ep.tile([P, H], f32, tag="den")
                nc.scalar.activation(out=den[:], in_=acc[:, D:],
                                     func=mybir.ActivationFunctionType.Copy,
                                     bias=float(EPS))
                recip = ep.tile([P, H], f32, tag="recip")
                nc.vector.reciprocal(recip[:], den[:])
                out_sb = ep.tile([P, D], f32, tag="outsb")
                nc.vector.tensor_tensor(
                    out=out_sb[:].rearrange("p (h r) -> p r h", h=H),
                    in0=acc[:, :D].rearrange("p (r h) -> p r h", h=H),
                    in1=recip[:].unsqueeze(1).to_broadcast([P, 32, H]),
                    op=mybir.AluOpType.mult)
                if with_bias:
                    nc.vector.tensor_tensor(
                        out=out_sb[:], in0=out_sb[:], in1=bias_mat[:],
                        op=mybir.AluOpType.add)
                nc.sync.dma_start(out_d[b * P:(b + 1) * P, :], out_sb[:])

            def proj_pass(which, xslabs):
                nt0, ntn, pt = ((0, NT_A, pt_a) if which == 0
                                else (NT_A, NT, pt_b))
                for ws in range(nt0 // WSLAB, cdiv(ntn, WSLAB)):
                    base = ws * WSLAB
                    w = min(WSLAB, ntn - base)
                    xt = xslabs[ws]
                    prow = p1w.tile([P, WSLAB * D], bf16, tag="prow")
                    prow_r = prow[:].rearrange("p (j c) -> p j c", j=WSLAB)
                    for g0 in range(0, w, PK):
                        k = min(PK, w - g0)
                        ps = p1ps.tile([P, PK * 256], f32, tag="ps")
                        for j in range(k):
                            nt = base + g0 + j
                            o = (nt % XSLAB) * P
                            nc.tensor.matmul(out=ps[:, j * 256:j * 256 + D],
                                             lhsT=xt[:, o:o + P],
                                             rhs=W_sb[:], start=True, stop=True)
                        ps_r = ps[:].rearrange("p (j c) -> p j c", j=PK)[:, :k, :]
                        nc.scalar.activation(
                            out=prow_r[:, g0:g0 + k, :], in_=ps_r[:, :, 0:D],
                            func=mybir.ActivationFunctionType.Copy)
                    nc.gpsimd.dma_start(
                        pt[:].bitcast(bf16).rearrange(
                            "(p nt) c -> p nt c",
                            p=P)[:, base - nt0:base - nt0 + w, :],
                        prow_r[:, :w, :])

            # ---------- emission: earlies + pass A + lates + pass B ----------
            staged = {}
            for b in range(min(LAG, NBLK)):
                staged[b] = early(b)

            # x slabs were consumed by the score pass; reload for proj passes.
            xs2 = [load_slab(i) for i in range(NXS)]
            proj_pass(0, xs2)

            nxt = LAG
            for b in range(NBLK):
                Sg, wide = staged.pop(b)
                acc = accp.tile([P, D + H], f32, tag="acc")
                late_half(b, Sg, wide, acc, 0)
                if nxt < NBLK:
                    staged[nxt] = early(nxt)
                    nxt += 1
                staged[b] = (Sg, wide, acc)
                if b == LAG - 1:
                    proj_pass(1, xs2)
            for b in range(NBLK):
                Sg, wide, acc = staged.pop(b)
                late_half(b, Sg, wide, acc, 1)
                epilogue(b, acc)

    nc.compile()
    return nc


def _wrap16(seg):
    """dma_gather idx layout: entry i at [i%16, i//16], replicated x8."""
    n = len(seg)
    w = seg.reshape(n // 16, 16).T
    return np.tile(w, (8, 1))


def _prep_host(in_feat, edge_ind, W_proj, a_src, a_tgt, bias):
    import ml_dtypes
    bfd = ml_dtypes.bfloat16
    src = np.asarray(edge_ind[0]).astype(np.int64)
    tgt = np.asarray(edge_ind[1]).astype(np.int64)
    x = np.asarray(in_feat, np.float32)
    W = np.asarray(W_proj, np.float32)
    a_src = np.asarray(a_src, np.float32).reshape(H, 32)
    a_tgt = np.asarray(a_tgt, np.float32).reshape(H, 32)
    bias = np.asarray(bias, np.float32).reshape(-1)

    # W head-interleaved (col r*4+h); WA = [W@a_src_h | W@a_tgt_h]
    Wb = W.astype(bfd).astype(np.float32)
    perm = np.arange(D).reshape(H, 32).T.reshape(-1)
    W_in = Wb[:, perm]
    WA = np.zeros((P, 8), np.float32)
    for h in range(H):
        sel = np.zeros((D,), np.float32)
        sel[h * 32:(h + 1) * 32] = a_src[h]
        WA[:, h] = Wb @ sel
        sel = np.zeros((D,), np.float32)
        sel[h * 32:(h + 1) * 32] = a_tgt[h]
        WA[:, H + h] = Wb @ sel

    xT = np.zeros((P, NPAD), np.float32)
    xT[:, :N_NODES] = x.T
    xs_in = {}
    for i in range(NXS):
        sl = np.zeros((P, XSLAB * P), bfd)
        w = min(XSLAB * P, NPAD - i * XSLAB * P)
        sl[:, :w] = xT[:, i * XSLAB * P:i * XSLAB * P + w].astype(bfd)
        xs_in[f"xs{i}"] = sl

    ident = np.zeros((144, P), bfd)
    for q in range(P):
        ident[q, q] = 1.0

    # ---- edge partitioning ----
    # target shard: tile-aligned per-core ranges
    tile_core = np.minimum((np.arange(NT) * N_CORES) // NT, N_CORES - 1)
    core_of_node = tile_core[np.arange(N_NODES) // P]
    core = core_of_node[tgt]
    p_of_t = tgt % P
    t_is_lo = p_of_t < PSPLIT
    src_is_lo = (src % P) < PSPLIT
    src_is_a = (src // P) < NT_A

    deg4 = np.zeros((4, N_NODES), np.int64)
    seg_src = (~src_is_a).astype(np.int64) * 2 + (~src_is_lo).astype(np.int64)
    for s0 in range(4):
        deg4[s0] = np.bincount(tgt[seg_src == s0], minlength=N_NODES)
    deg_a = deg4[0] + deg4[1]
    deg_b = deg4[2] + deg4[3]
    blk_of = np.full(N_NODES, -1, np.int32)
    tin_of = np.zeros(N_NODES, np.int32)
    nb_lo = nb_hi = 0
    for c in range(N_CORES):
        ids_all = np.nonzero(core_of_node == c)[0]
        nb_lo = max(nb_lo, cdiv(int(((ids_all % P) < PSPLIT).sum()), P))
        nb_hi = max(nb_hi, cdiv(int(((ids_all % P) >= PSPLIT).sum()), P))
    for c in range(N_CORES):
        ids_all = np.nonzero(core_of_node == c)[0]
        for half, nb, b0 in ((0, nb_lo, 0), (1, nb_hi, nb_lo)):
            sel = (ids_all % P) < PSPLIT if half == 0 else (ids_all % P) >= PSPLIT
            ids = ids_all[sel]
            order = np.argsort(-(deg_a[ids] + deg_b[ids]), kind="stable")
            loads = np.zeros((4, nb), np.int64)
            fill = np.zeros(nb, np.int32)
            for t in ids[order]:
                cand = np.nonzero(fill < P)[0]
                j = cand[np.argmin((loads[:, cand] + deg4[:, t:t + 1]).max(0)
                                   + 0.001 * fill[cand])]
                blk_of[t] = b0 + j
                tin_of[t] = fill[j]
                fill[j] += 1
                loads[:, j] += deg4[:, t]
    NBLK = nb_lo + nb_hi
    blk = blk_of[tgt]
    tin = tin_of[tgt]

    # per (core, block) per segment (A-lo, A-hi, B-lo, B-hi) tile counts
    seg_of = (~src_is_a).astype(np.int64) * 2 + (~src_is_lo).astype(np.int64)
    key = (core * NBLK + blk) * 4 + seg_of
    seg_n = np.bincount(key, minlength=N_CORES * NBLK * 4).reshape(-1, 4)
    ks = tuple(max(1, cdiv(int(seg_n[:, s].max()), P)) for s in range(4))
    k_alo, k_ahi, k_blo, k_bhi = ks
    T_B = sum(ks)
    IW = T_B * 8
    IWPAD = cdiv(NBLK * IW * 2, 256) * 128

    # row ids: proj tables by pass; score tables by p-half.
    prow_id = np.where(src_is_a, (src % P) * NT_A + src // P,
                       (src % P) * NT_B + (src // P - NT_A))
    crow_id = (src % P - np.where(src_is_lo, 0, PSPLIT)) * NT + src // P
    trow_id = (tgt % P - np.where(t_is_lo, 0, PSPLIT)) * NT + tgt // P

    seq = _wrap16(np.concatenate([np.arange(P, dtype=np.int16),
                                  np.zeros(P, np.int16)]))[:, :16]
    with_bias = bool(np.any(bias != 0.0))
    shared = {**xs_in, "W": W_in.astype(bfd), "WA": WA.astype(bfd),
              "ident": ident.view(np.int64), "seq": seq}
    if with_bias:
        shared["bias"] = bias.reshape(1, D)

    bounds = np.cumsum([0, k_alo, k_ahi, k_blo, k_bhi]) * P
    core_inputs = []
    out_perm = np.full((N_CORES, NBLK * P), -1, np.int64)
    for c in range(N_CORES):
        ids_all = np.nonzero(core_of_node == c)[0]
        for t in ids_all:
            out_perm[c, blk_of[t] * P + tin_of[t]] = t
        m = core == c
        cb = blk[m]
        cseg = seg_of[m]
        cp_, cc, ct_, cti = prow_id[m], crow_id[m], trow_id[m], tin[m]
        pidx = np.zeros((NBLK, T_B * P), np.int16)
        c16 = np.zeros((NBLK, T_B * P), np.int16)
        t16 = np.zeros((NBLK, T_B * P), np.int16)
        s16 = np.full((NBLK, T_B * P), 128, np.int16)
        for b in range(NBLK):
            mb = cb == b
            for s0 in range(4):
                ms = mb & (cseg == s0)
                n = int(ms.sum())
                o = int(bounds[s0])
                pidx[b, o:o + n] = cp_[ms].astype(np.int16)
                c16[b, o:o + n] = cc[ms].astype(np.int16)
                t16[b, o:o + n] = ct_[ms].astype(np.int16)
                s16[b, o:o + n] = cti[ms].astype(np.int16)
        pw = np.zeros((P, IWPAD), np.int16)
        cw = np.zeros((P, IWPAD), np.int16)
        tw = np.zeros((P, IWPAD), np.int16)
        sw_ = np.zeros((P, IWPAD), np.int16)
        for b in range(NBLK):
            for s0 in range(4):
                lo, hi = bounds[s0], bounds[s0 + 1]
                pw[:, b * IW + lo // 16:b * IW + hi // 16] = \
                    _wrap16(pidx[b, lo:hi])
                cw[:, b * IW + lo // 16:b * IW + hi // 16] = \
                    _wrap16(c16[b, lo:hi])
            tw[:, b * IW:(b + 1) * IW] = _wrap16(t16[b])
            sw_[:, b * IW:(b + 1) * IW] = _wrap16(s16[b])
        core_inputs.append({**shared, "pidx": pw, "cidx": cw,
                            "tidx": tw, "sidx": sw_})
    return (nb_lo, nb_hi, ks, with_bias), core_inputs, out_perm


def kernel(in_feat, edge_ind, edge_len, W_proj, a_src, a_tgt, bias):
    kkey, core_inputs, out_perm = _prep_host(in_feat, edge_ind, W_proj,
                                             a_src, a_tgt, bias)
    if kkey not in _cache:
        _cache[kkey] = _build(*kkey)
    nc = _cache[kkey]

    from concourse.bass_utils import run_bass_kernel_spmd
    res = run_bass_kernel_spmd(nc, core_inputs, list(range(N_CORES)))

    out = np.zeros((N_NODES, D), np.float32)
    for c in range(N_CORES):
        o = res.results[c]["out"]
        valid = out_perm[c] >= 0
        out[out_perm[c][valid]] = o[valid]
    return out
